# revision 1
# baseline (speedup 1.0000x reference)
"""Trainium2 Bass kernel for nn_Decoder (LSTM decoder + attention + copy mechanism).

Strategy: pure batch-parallel across the 8 NeuronCores — each core runs the
full T=48-step recurrence and the vocab projection for its 4 batch elements,
with zero cross-core communication (this runtime exposes none). Weights for
the four big gate matrices are SBUF-resident in fp8(e3m4); activations and
the output-side matrices (Wc, Wp) are bf16; all accumulation is fp32.

Self-contained: builds the Bass program, shards inputs on the host, runs via
run_bass_kernel_spmd on cores 0-7, reassembles the full [T, B, V] output.
"""
import sys

sys.path.insert(0, "/opt/trn_rl_repo")

import numpy as np
import ml_dtypes

import concourse.bass as bass
import concourse.mybir as mybir
import concourse.tile as tile
from concourse.bass_utils import run_bass_kernel_spmd

F32 = mybir.dt.float32
BF16 = mybir.dt.bfloat16
FP8 = mybir.dt.float8e3
I16 = mybir.dt.int16
AF = mybir.ActivationFunctionType
ALU = mybir.AluOpType

nbf16 = ml_dtypes.bfloat16
nfp8 = ml_dtypes.float8_e3m4

V, E, H = 10000, 512, 1024
T, S, B = 48, 48, 32
PAD, COPY_ID, EPS = 0, 1, 1e-7
NCORES = 8
BL = B // NCORES            # batch per core = 4
G4 = 4 * H                  # 4096 gate width
NVC = 20                    # vocab chunks of 512 (last chunk 10000-19*512=272... 20*512=10240>V)
VCH = 512
KC_E = E // 128             # 4
KC_H = H // 128             # 8
VKC = (V + 127) // 128      # 79 chunks over vocab for the embed gather


def _ceil(a, b):
    return (a + b - 1) // b


# ---------------------------------------------------------------- wait split
def _split_wide_waits(nc):
    """walrus CTRL codegen accepts at most 1 sync-wait per instruction; move
    excess waits onto preceding NoOps on the same (in-order) engine."""
    for f in nc.m.functions:
        for bb in f.blocks:
            ins_list = list(bb.instructions)
            out = []
            changed = False
            for ins in ins_list:
                si = getattr(ins, "sync_info", None)
                waits = list(si.on_wait) if si is not None else []
                if len(waits) > 1:
                    excess, keep = waits[:-1], waits[-1:]
                    for w in excess:
                        nop = mybir.InstNoOp(
                            name=f"I-{nc.next_id()}",
                            opcode="NoOp",
                            engine=ins.engine,
                            debug=ins.debug,
                            ins=[],
                            outs=[],
                            sync_info=mybir.SyncInfo(on_wait=[w], on_update=[]),
                        )
                        try:
                            nc.register_instruction(nop, overwrite=True)
                        except Exception:
                            pass
                        out.append(nop)
                        changed = True
                    si.on_wait = keep
                    ins.sync_info = si
                out.append(ins)
            if changed:
                try:
                    bb.instructions = out
                except Exception:
                    bb.instructions.clear()
                    bb.instructions.extend(out)


# ---------------------------------------------------------------- program
def build_program(t_steps=T):
    nc = bass.Bass("TRN2")
    dp = nc.declare_dram_parameter

    w0f8_d = dp("w0f8", [KC_H, 128, G4], FP8, isOutput=False)   # W_ih0[:,E:]^T
    wh08_d = dp("wh08", [KC_H, 128, G4], FP8, isOutput=False)   # W_hh0^T
    wi18_d = dp("wi18", [KC_H, 128, G4], FP8, isOutput=False)   # W_ih1^T
    wh18_d = dp("wh18", [KC_H, 128, G4], FP8, isOutput=False)   # W_hh1^T
    wcb_d = dp("wcb", [2 * KC_H, 128, H], BF16, isOutput=False)  # Wc^T
    wpb_d = dp("wpb", [KC_H, 128, NVC * VCH], BF16, isOutput=False)  # Wp^T padded
    we0b_d = dp("we0b", [KC_E, 128, G4], BF16, isOutput=False)  # W_ih0[:,:E]^T
    wkTb_d = dp("wkTb", [KC_H, 128, H], BF16, isOutput=False)   # Wk^T
    embed_d = dp("embed_bf", [VKC * 128, E], BF16, isOutput=False)  # padded rows
    encIA_d = dp("encIA", [128, H], BF16, isOutput=False)  # enc rows (s*4+b), s<32
    encIB_d = dp("encIB", [64, H], BF16, isOutput=False)   # s in 32..47
    encT_d = dp("encT", [KC_H, 128, BL * S], BF16, isOutput=False)  # [hchunk, (b,s)]
    reftok_d = dp("reftok", [128, t_steps * BL], F32, isOutput=False)
    vidx_d = dp("vidx", [128, VKC], F32, isOutput=False)        # p + 128*ch
    iota512_d = dp("iota512", [128, VCH], F32, isOutput=False)
    srcsh_d = dp("srcsh", [128, 2 * NVC], F32, isOutput=False)  # interleaved (s*4+b)
    pen_d = dp("pen", [BL, S * BL], F32, isOutput=False)        # full (s,b) penalty
    h0T_d = dp("h0T", [128, KC_H * BL], BF16, isOutput=False)
    h1T_d = dp("h1T", [128, KC_H * BL], BF16, isOutput=False)
    c0_d = dp("c0i", [BL, H], F32, isOutput=False)
    c1_d = dp("c1i", [BL, H], F32, isOutput=False)
    ident4_d = dp("ident4", [4, 4], BF16, isOutput=False)

    y_d = dp("y", [t_steps, BL, V], F32, isOutput=True)

    NR = t_steps * BL
    mtiles = [(r0, min(128, NR - r0)) for r0 in range(0, NR, 128)]

    with tile.TileContext(nc) as tc:
        import contextlib
        _stack = contextlib.ExitStack()
        with tc.tile_pool(name="wres", bufs=1) as wpool, \
             tc.tile_pool(name="dram", bufs=1, space="DRAM") as dpool:

            eg_dram = dpool.tile([t_steps * BL, G4], BF16, name="eg_dram")
            oh_dram = dpool.tile([192, NVC * VCH], BF16, name="oh_dram")
            e_dram = dpool.tile([t_steps * BL, NVC * VCH], BF16, name="e_dram")

            dma = nc.sync.dma_start

            # ---- resident
            w0f = wpool.tile([128, KC_H * G4], FP8, name="w0f")
            wh0 = wpool.tile([128, KC_H * G4], FP8, name="wh0")
            wi1 = wpool.tile([128, KC_H * G4], FP8, name="wi1")
            wh1 = wpool.tile([128, KC_H * G4], FP8, name="wh1")
            wcb = wpool.tile([128, 2 * KC_H * H], BF16, name="wcb")
            encIA = wpool.tile([128, H], BF16, name="encIA")
            encIB = wpool.tile([64, H], BF16, name="encIB")
            attKT = wpool.tile([128, KC_H * BL * S], BF16, name="attKT")
            srcsh = wpool.tile([128, 2 * NVC], F32, name="srcsh")
            pen = wpool.tile([BL, S * BL], F32, name="pen")
            ident4 = wpool.tile([4, 4], BF16, name="ident4")
            h0T = wpool.tile([128, KC_H * BL], BF16, name="h0T")
            h1T = wpool.tile([128, KC_H * BL], BF16, name="h1T")
            c0 = wpool.tile([BL, H], F32, name="c0")
            c1 = wpool.tile([BL, H], F32, name="c1")
            combT0 = wpool.tile([128, KC_H * BL], BF16, name="combT0")
            combT = wpool.tile([128, KC_H * NR], BF16, name="combT")
            dsbA = wpool.tile([128, NR], BF16, name="dsbA")
            dsbB = wpool.tile([64, NR], BF16, name="dsbB")
            sumT = wpool.tile([128, KC_H * BL], BF16, name="sumT")
            zbuf = wpool.tile([128, 2 * NVC], F32, name="zbuf")
            cwn = wpool.tile([128, 2], F32, name="cwn")
            cw = wpool.tile([128, 2], F32, name="cw")
            spp = wpool.tile([128, 2], F32, name="spp")
            ceps = wpool.tile([128, 2], F32, name="ceps")

            for dst, srct in ((w0f, w0f8_d), (wh0, wh08_d), (wi1, wi18_d),
                              (wh1, wh18_d)):
                for k in range(KC_H):
                    dma(out=dst[:, k * G4:(k + 1) * G4], in_=srct[k])
            for k in range(2 * KC_H):
                dma(out=wcb[:, k * H:(k + 1) * H], in_=wcb_d[k])
            dma(out=encIA[:], in_=encIA_d[:])
            dma(out=encIB[:], in_=encIB_d[:])
            dma(out=srcsh[:], in_=srcsh_d[:])
            dma(out=pen[:], in_=pen_d[:])
            dma(out=ident4[:], in_=ident4_d[:])
            dma(out=h0T[:], in_=h0T_d[:])
            dma(out=h1T[:], in_=h1T_d[:])
            dma(out=c0[:], in_=c0_d[:])
            dma(out=c1[:], in_=c1_d[:])
            nc.vector.memset(combT0[:], 0.0)

            # ======== phase 0 (scoped pool, freed afterwards)
            with tc.tile_pool(name="ph0", bufs=1) as p0, \
                 tc.tile_pool(name="ps0", bufs=1, space="PSUM") as ps0:
                reftok = p0.tile([128, NR], F32, name="reftok")
                vidx = p0.tile([128, VKC], F32, name="vidx")
                iota512 = p0.tile([128, VCH], F32, name="iota512")
                XeT = p0.tile([128, KC_E * NR], BF16, name="XeT")
                dma(out=reftok[:], in_=reftok_d[:])
                dma(out=vidx[:], in_=vidx_d[:])
                dma(out=iota512[:], in_=iota512_d[:])

                # 0a: X_embT = embed^T @ onehot(ref_tokens)
                psX = [ps0.tile([128, NR], F32, name=f"psX{m}", tag=f"psX{m}",
                                bufs=1) for m in range(KC_E)]
                for ch in range(VKC):
                    oref = p0.tile([128, NR], BF16, name="oref", tag="oref", bufs=4)
                    nc.vector.tensor_scalar(out=oref[:], in0=reftok[:],
                                            scalar1=vidx[:, ch:ch + 1], scalar2=None,
                                            op0=ALU.is_equal)
                    emb = p0.tile([128, E], BF16, name="emb", tag="emb", bufs=6)
                    dma(out=emb[:], in_=embed_d[ch * 128:(ch + 1) * 128, :])
                    for m in range(KC_E):
                        nc.tensor.matmul(psX[m][:], lhsT=emb[:, m * 128:(m + 1) * 128],
                                         rhs=oref[:], start=(ch == 0),
                                         stop=(ch == VKC - 1))
                for m in range(KC_E):
                    nc.vector.tensor_copy(out=XeT[:, m * NR:(m + 1) * NR],
                                          in_=psX[m][:])

                # 0b: Eg -> eg_dram [(t,b), 4H]
                for mt, (r0, mm) in enumerate(mtiles):
                    for n in range(8):
                        pse = ps0.tile([128, 512], F32, name="pse", tag="pse",
                                       bufs=2)
                        for k in range(KC_E):
                            wck = p0.tile([128, 512], BF16, name="wck", tag="wck",
                                          bufs=6)
                            dma(out=wck[:], in_=we0b_d[k, :, n * 512:(n + 1) * 512])
                            nc.tensor.matmul(pse[:mm, :],
                                             lhsT=XeT[:, k * NR + r0:k * NR + r0 + mm],
                                             rhs=wck[:], start=(k == 0),
                                             stop=(k == KC_E - 1))
                        egs = p0.tile([128, 512], BF16, name="egs", tag="egs", bufs=2)
                        nc.vector.tensor_copy(out=egs[:mm, :], in_=pse[:mm, :])
                        dma(out=eg_dram[r0:r0 + mm, n * 512:(n + 1) * 512],
                            in_=egs[:mm, :])

                # 0c: att_keyT = Wk @ enc^T
                ects = []
                for k in range(KC_H):
                    ecx = p0.tile([128, BL * S], BF16, name=f"ect{k}")
                    dma(out=ecx[:], in_=encT_d[k])
                    ects.append(ecx)
                for mt in range(KC_H):
                    psa = ps0.tile([128, BL * S], F32, name="psa", tag="pse",
                                   bufs=2)
                    for k in range(KC_H):
                        wkc = p0.tile([128, 128], BF16, name="wkc", tag="wkc", bufs=6)
                        dma(out=wkc[:], in_=wkTb_d[k, :, mt * 128:(mt + 1) * 128])
                        nc.tensor.matmul(psa[:], lhsT=wkc[:], rhs=ects[k][:],
                                         start=(k == 0), stop=(k == KC_H - 1))
                    nc.vector.tensor_copy(out=attKT[:, mt * BL * S:(mt + 1) * BL * S],
                                          in_=psa[:])

                # 0d: onehot tiles (interleaved rows s*4+b) -> oh_dram
                for tl, nrow in ((0, 128), (1, 64)):
                    for ch in range(NVC):
                        oh = p0.tile([128, VCH], BF16, name="oh", tag="oh", bufs=2)
                        nc.vector.tensor_scalar(
                            out=oh[:nrow, :], in0=iota512[:nrow, :],
                            scalar1=srcsh[:nrow, tl * NVC + ch:tl * NVC + ch + 1],
                            scalar2=None, op0=ALU.is_equal)
                        dma(out=oh_dram[tl * 128:tl * 128 + nrow,
                                        ch * VCH:(ch + 1) * VCH], in_=oh[:nrow, :])

            # ======== phase 1
            pspool = _stack.enter_context(
                tc.tile_pool(name="ps", bufs=2, space="PSUM"))
            SIG, TANH = AF.Sigmoid, AF.Tanh
            with tc.tile_pool(name="ph1", bufs=1) as p1:
                for t in range(t_steps):
                    for layer in range(2):
                        wx, wh = (w0f, wh0) if layer == 0 else (wi1, wh1)
                        hT_prev = h0T if layer == 0 else h1T
                        cst = c0 if layer == 0 else c1
                        hb = p1.tile([BL, H], BF16, name="hb", tag="hb", bufs=2)
                        for half in range(2):
                            ga = p1.tile([BL, 2048], F32, name="ga", tag="ga", bufs=1)
                            # chunks n = half, half+2, half+4, half+6 (i,f,g,o slices)
                            for gi, n in enumerate(range(half, 8, 2)):
                                psg = pspool.tile([BL, 512], F32, name="psg",
                                                  tag="psg", bufs=2)
                                first = True
                                egt = None
                                if layer == 0:
                                    egt = p1.tile([BL, 512], BF16, name="egt",
                                                  tag="egt", bufs=3)
                                    dma(out=egt[:],
                                        in_=eg_dram[t * BL:(t + 1) * BL,
                                                    n * 512:(n + 1) * 512])
                                for k in range(KC_H):
                                    if layer == 0:
                                        lh = (combT0[:, k * BL:(k + 1) * BL] if t == 0
                                              else combT[:, k * NR + (t - 1) * BL:
                                                         k * NR + t * BL])
                                    else:
                                        lh = h0T[:, k * BL:(k + 1) * BL]
                                    nc.tensor.matmul(
                                        psg[:], lhsT=lh,
                                        rhs=wx[:, k * G4 + n * 512:
                                               k * G4 + (n + 1) * 512],
                                        start=first, stop=False)
                                    first = False
                                for k in range(KC_H):
                                    nc.tensor.matmul(
                                        psg[:], lhsT=hT_prev[:, k * BL:(k + 1) * BL],
                                        rhs=wh[:, k * G4 + n * 512:
                                               k * G4 + (n + 1) * 512],
                                        start=False, stop=(k == KC_H - 1))
                                if egt is not None:
                                    gadd = p1.tile([BL, 512], F32, name="gadd",
                                                   tag="gadd", bufs=2)
                                    nc.vector.tensor_tensor(out=gadd[:], in0=psg[:],
                                                            in1=egt[:], op=ALU.add)
                                    asrc = gadd
                                else:
                                    asrc = psg
                                nc.scalar.activation(
                                    out=ga[:, gi * 512:(gi + 1) * 512], in_=asrc[:],
                                    func=(TANH if gi == 2 else SIG))
                            # half c/h update: ga = [i, f, g, o] for h-cols hc
                            hc = slice(half * 512, half * 512 + 512)
                            t2 = p1.tile([BL, 512], F32, name="t2", tag="t2", bufs=2)
                            nc.vector.tensor_tensor(out=cst[:, hc], in0=ga[:, 512:1024],
                                                    in1=cst[:, hc], op=ALU.mult)
                            nc.vector.tensor_tensor(out=t2[:], in0=ga[:, 0:512],
                                                    in1=ga[:, 1024:1536], op=ALU.mult)
                            nc.vector.tensor_tensor(out=cst[:, hc], in0=cst[:, hc],
                                                    in1=t2[:], op=ALU.add)
                            th = p1.tile([BL, 512], F32, name="th", tag="t2", bufs=2)
                            nc.scalar.activation(out=th[:], in_=cst[:, hc], func=TANH)
                            nc.vector.tensor_tensor(out=hb[:, hc], in0=ga[:, 1536:2048],
                                                    in1=th[:], op=ALU.mult)
                        # transpose h -> hT (written AFTER all reads of prev value)
                        hT_new = h0T if layer == 0 else h1T
                        for k in range(KC_H):
                            psT = pspool.tile([128, BL], BF16, name="psT", tag="psT",
                                              bufs=2)
                            nc.tensor.transpose(psT[:], hb[:, k * 128:(k + 1) * 128],
                                                ident4[:])
                            nc.vector.tensor_copy(out=hT_new[:, k * BL:(k + 1) * BL],
                                                  in_=psT[:])

                    # ---- attention
                    pss = pspool.tile([BL, BL * S], F32, name="pss", tag="pss", bufs=1)
                    for k in range(KC_H):
                        nc.tensor.matmul(pss[:], lhsT=h1T[:, k * BL:(k + 1) * BL],
                                         rhs=attKT[:, k * BL * S:(k + 1) * BL * S],
                                         start=(k == 0), stop=(k == KC_H - 1))
                    nc.vector.tensor_tensor(out=pss[:], in0=pss[:], in1=pen[:],
                                            op=ALU.add)
                    ssum = p1.tile([BL, 1], F32, name="ssum", tag="ssum", bufs=2)
                    dstc = p1.tile([BL, S * BL], F32, name="dstc", tag="dstc", bufs=1)
                    nc.scalar.activation(out=dstc[:], in_=pss[:], func=AF.Exp,
                                         accum_out=ssum[:])
                    rs = p1.tile([BL, 1], F32, name="rs", tag="ssum", bufs=2)
                    nc.vector.reciprocal(out=rs[:], in_=ssum[:])
                    dstb = p1.tile([BL, S * BL], BF16, name="dstb", tag="dstb2", bufs=1)
                    nc.vector.tensor_scalar(out=dstb[:], in0=dstc[:], scalar1=rs[:],
                                            scalar2=None, op0=ALU.mult)
                    psDA = pspool.tile([128, BL], BF16, name="psDA", tag="psT", bufs=2)
                    nc.tensor.transpose(psDA[:], dstb[:, 0:128], ident4[:])
                    nc.vector.tensor_copy(out=dsbA[:, t * BL:(t + 1) * BL],
                                          in_=psDA[:])
                    psDB = pspool.tile([64, BL], BF16, name="psDB", tag="psT", bufs=2)
                    nc.tensor.transpose(psDB[:], dstb[:, 128:192], ident4[:])
                    nc.vector.tensor_copy(out=dsbB[:, t * BL:(t + 1) * BL],
                                          in_=psDB[:])

                    # summary via block-sparse dist: out[h, b] per h-chunk
                    pssu = pspool.tile([128, KC_H * BL], F32, name="pssu", tag="pss",
                                       bufs=1)
                    for j in range(KC_H):
                        nc.tensor.matmul(
                            pssu[:, j * BL:(j + 1) * BL],
                            lhsT=encIA[:, j * 128:(j + 1) * 128],
                            rhs=dsbA[:, t * BL:(t + 1) * BL],
                            start=True, stop=False)
                        nc.tensor.matmul(
                            pssu[:, j * BL:(j + 1) * BL],
                            lhsT=encIB[:, j * 128:(j + 1) * 128],
                            rhs=dsbB[:, t * BL:(t + 1) * BL],
                            start=False, stop=True)
                    nc.vector.tensor_copy(out=sumT[:], in_=pssu[:])

                    # comb -> combT col block t
                    cbb = p1.tile([BL, H], BF16, name="cbb", tag="hb", bufs=2)
                    for n in range(2):
                        psc = pspool.tile([BL, 512], F32, name="psc", tag="psg",
                                          bufs=2)
                        for k in range(KC_H):
                            nc.tensor.matmul(
                                psc[:], lhsT=h1T[:, k * BL:(k + 1) * BL],
                                rhs=wcb[:, k * H + n * 512:k * H + (n + 1) * 512],
                                start=(k == 0), stop=False)
                        for k in range(KC_H):
                            nc.tensor.matmul(
                                psc[:], lhsT=sumT[:, k * BL:(k + 1) * BL],
                                rhs=wcb[:, (KC_H + k) * H + n * 512:
                                         (KC_H + k) * H + (n + 1) * 512],
                                start=False, stop=(k == KC_H - 1))
                        nc.vector.tensor_copy(out=cbb[:, n * 512:(n + 1) * 512],
                                              in_=psc[:])
                    for k in range(KC_H):
                        psT2 = pspool.tile([128, BL], BF16, name="psT2", tag="psT",
                                           bufs=2)
                        nc.tensor.transpose(psT2[:], cbb[:, k * 128:(k + 1) * 128],
                                            ident4[:])
                        nc.vector.tensor_copy(
                            out=combT[:, k * NR + t * BL:k * NR + (t + 1) * BL],
                            in_=psT2[:])

            # ======== phase 2 (own pool); vc outer so Wp/onehot stream once
            with tc.tile_pool(name="ph2", bufs=1) as p2:
                for vc in range(NVC):
                    vlim = min(VCH, V - vc * VCH)
                    wpcs = []
                    for k in range(KC_H):
                        wpc = p2.tile([128, VCH], BF16, name="wpc", tag=f"wpc{k}",
                                      bufs=2)
                        dma(out=wpc[:], in_=wpb_d[k, :, vc * VCH:(vc + 1) * VCH])
                        wpcs.append(wpc)
                    for mt, (r0, mm) in enumerate(mtiles):
                        psp = pspool.tile([128, VCH], F32, name="psp", tag="psg",
                                          bufs=2)
                        for k in range(KC_H):
                            nc.tensor.matmul(
                                psp[:mm, :],
                                lhsT=combT[:, k * NR + r0:k * NR + r0 + mm],
                                rhs=wpcs[k][:], start=(k == 0), stop=(k == KC_H - 1))
                        esb = p2.tile([128, VCH], BF16, name="esb", tag="esb", bufs=3)
                        nc.scalar.activation(out=esb[:mm, :vlim], in_=psp[:mm, :vlim],
                                             func=AF.Exp,
                                             accum_out=zbuf[:mm, mt * NVC + vc:
                                                            mt * NVC + vc + 1])
                        if vc == 0:
                            nc.scalar.activation(out=cwn[:mm, mt:mt + 1],
                                                 in_=psp[:mm, COPY_ID:COPY_ID + 1],
                                                 func=AF.Exp)
                        dma(out=e_dram[r0:r0 + mm, vc * VCH:vc * VCH + vlim],
                            in_=esb[:mm, :vlim])
                for mt, (r0, mm) in enumerate(mtiles):
                    zt = p2.tile([128, 1], F32, name="zt", tag="zt", bufs=2)
                    nc.vector.tensor_reduce(out=zt[:mm, :],
                                            in_=zbuf[:mm, mt * NVC:(mt + 1) * NVC],
                                            op=ALU.add, axis=mybir.AxisListType.X)
                    iz = p2.tile([128, 1], F32, name="iz", tag="zt", bufs=2)
                    nc.vector.reciprocal(out=iz[:mm, :], in_=zt[:mm, :])
                    nc.vector.tensor_tensor(out=cw[:mm, mt:mt + 1],
                                            in0=cwn[:mm, mt:mt + 1], in1=iz[:mm, :],
                                            op=ALU.mult)
                    omc = p2.tile([128, 1], F32, name="omc", tag="zt", bufs=2)
                    nc.vector.tensor_scalar(out=omc[:mm, :], in0=cw[:mm, mt:mt + 1],
                                            scalar1=-1.0, scalar2=1.0,
                                            op0=ALU.mult, op1=ALU.add)
                    nc.vector.tensor_tensor(out=spp[:mm, mt:mt + 1], in0=omc[:mm, :],
                                            in1=iz[:mm, :], op=ALU.mult)
                    nc.vector.tensor_scalar(out=ceps[:mm, mt:mt + 1],
                                            in0=cw[:mm, mt:mt + 1],
                                            scalar1=EPS, scalar2=None, op0=ALU.mult)
                for vc in range(NVC):
                    vlim = min(VCH, V - vc * VCH)
                    ohA = p2.tile([128, VCH], BF16, name="ohA", tag="ohA", bufs=2)
                    dma(out=ohA[:, :vlim],
                        in_=oh_dram[0:128, vc * VCH:vc * VCH + vlim])
                    ohB = p2.tile([64, VCH], BF16, name="ohB", tag="ohB", bufs=2)
                    dma(out=ohB[:, :vlim],
                        in_=oh_dram[128:192, vc * VCH:vc * VCH + vlim])
                    for mt, (r0, mm) in enumerate(mtiles):
                        tm = mm // BL
                        e2 = p2.tile([128, VCH], BF16, name="e2", tag="esb", bufs=3)
                        dma(out=e2[:mm, :vlim],
                            in_=e_dram[r0:r0 + mm, vc * VCH:vc * VCH + vlim])
                        pscp = pspool.tile([128, VCH], F32, name="pscp", tag="psg",
                                           bufs=2)
                        nc.tensor.matmul(pscp[:mm, :vlim],
                                         lhsT=dsbA[:, r0:r0 + mm],
                                         rhs=ohA[:, :vlim], start=True, stop=False)
                        nc.tensor.matmul(pscp[:mm, :vlim],
                                         lhsT=dsbB[:, r0:r0 + mm],
                                         rhs=ohB[:, :vlim], start=False, stop=True)
                        nc.vector.tensor_scalar(out=pscp[:mm, :vlim],
                                                in0=pscp[:mm, :vlim],
                                                scalar1=cw[:mm, mt:mt + 1],
                                                scalar2=ceps[:mm, mt:mt + 1],
                                                op0=ALU.mult, op1=ALU.add)
                        ppf = p2.tile([128, VCH], F32, name="ppf", tag="ppf", bufs=2)
                        nc.vector.tensor_scalar(out=ppf[:mm, :vlim], in0=e2[:mm, :vlim],
                                                scalar1=spp[:mm, mt:mt + 1],
                                                scalar2=None, op0=ALU.mult)
                        nc.vector.tensor_tensor(out=ppf[:mm, :vlim],
                                                in0=ppf[:mm, :vlim],
                                                in1=pscp[:mm, :vlim], op=ALU.add)
                        outc = p2.tile([128, VCH], F32, name="outc", tag="ppf", bufs=2)
                        nc.scalar.activation(out=outc[:mm, :vlim], in_=ppf[:mm, :vlim],
                                             func=AF.Ln)
                        dma(out=y_d[r0 // BL:r0 // BL + tm, 0:BL,
                                    vc * VCH:vc * VCH + vlim],
                            in_=outc[:mm, 0:vlim])

            _stack.close()

    _split_wide_waits(nc)
    return nc


# ---------------------------------------------------------------- host prep
def _chunk_kT(w, dtype):
    """[K, N] -> [K//128, 128, N]"""
    K = w.shape[0]
    return np.ascontiguousarray(w.reshape(K // 128, 128, -1)).astype(dtype)


def prep_core_inputs(inputs, c, t_steps=T):
    ii = {k: np.asarray(v) for k, v in inputs.items()}
    Bc = list(range(c * BL, (c + 1) * BL))
    W_ih0, W_hh0 = ii["W_ih0"].astype(np.float32), ii["W_hh0"].astype(np.float32)
    W_ih1, W_hh1 = ii["W_ih1"].astype(np.float32), ii["W_hh1"].astype(np.float32)
    Wc, Wp, Wk = ii["Wc"].astype(np.float32), ii["Wp"].astype(np.float32), ii["Wk"].astype(np.float32)
    enc = ii["enc_features"].astype(np.float32)
    embed = ii["embed"].astype(np.float32)
    rt, st = ii["ref_tokens"], ii["src_tokens"]

    d = {}
    d["w0f8"] = _chunk_kT(W_ih0[:, E:].T, nfp8)
    d["wh08"] = _chunk_kT(W_hh0.T, nfp8)
    d["wi18"] = _chunk_kT(W_ih1.T, nfp8)
    d["wh18"] = _chunk_kT(W_hh1.T, nfp8)
    d["wcb"] = _chunk_kT(Wc.T, nbf16)
    wpT = np.zeros((H, NVC * VCH), np.float32)
    wpT[:, :V] = Wp.T
    d["wpb"] = _chunk_kT(wpT, nbf16)
    d["we0b"] = _chunk_kT(W_ih0[:, :E].T, nbf16)
    d["wkTb"] = _chunk_kT(Wk.T, nbf16)
    embp = np.zeros((VKC * 128, E), np.float32)
    embp[:V] = embed
    d["embed_bf"] = embp.astype(nbf16)
    # enc interleaved rows (s*4+b): tile A s<32, tile B s>=32
    encI = enc[:, Bc, :].reshape(S * BL, H)  # row s*BL+b
    d["encIA"] = np.ascontiguousarray(encI[0:128]).astype(nbf16)
    d["encIB"] = np.ascontiguousarray(encI[128:192]).astype(nbf16)
    # encT: [hchunk, 128, (s, b)] s-major interleaved
    encT = enc[:, Bc, :].transpose(2, 0, 1).reshape(H, S * BL)
    d["encT"] = _chunk_kT(encT, nbf16)
    # reftok replicated: col (t*BL + b)
    rtc = rt[:t_steps][:, Bc].astype(np.float32).reshape(t_steps * BL)
    d["reftok"] = np.tile(rtc[None, :], (128, 1)).astype(np.float32)
    d["vidx"] = (np.arange(128)[:, None] + 128 * np.arange(VKC)[None, :]).astype(np.float32)
    d["iota512"] = np.tile(np.arange(VCH, dtype=np.float32)[None, :], (128, 1))
    # srcsh [128, 2*NVC]: interleaved rows (s*4+b); tile 0: s<32, tile 1: s>=32
    stI = st[:, Bc].reshape(S * BL).astype(np.float32)  # row s*4+b
    srcsh = np.zeros((128, 2 * NVC), np.float32)
    for ch in range(NVC):
        srcsh[:, ch] = stI[0:128] - VCH * ch
        srcsh[0:64, NVC + ch] = stI[128:192] - VCH * ch
    d["srcsh"] = srcsh
    # pen_full [4, (s*4+b)]: row bp, col (s,b): -99999*mask if b==bp else -99999
    penf = np.full((BL, S * BL), -99999.0, np.float32)
    for bp in range(BL):
        penf[bp, bp::BL] = -99999.0 * (st[:, Bc[bp]] == PAD).astype(np.float32)
    d["pen"] = penf
    h0 = ii["h0"].astype(np.float32)
    c0 = ii["c0"].astype(np.float32)
    for li, name in ((0, "h0T"), (1, "h1T")):
        hT = h0[li][Bc].T  # [H, BL]
        d[name] = np.ascontiguousarray(
            hT.reshape(KC_H, 128, BL).transpose(1, 0, 2).reshape(128, KC_H * BL)
        ).astype(nbf16)
    d["c0i"] = c0[0][Bc].copy()
    d["c1i"] = c0[1][Bc].copy()
    d["ident4"] = np.eye(4, dtype=nbf16)
    # biases must be zero for this kernel (spec fill=zeros)
    for bn in ("bk", "bc", "bp", "b_ih0", "b_hh0", "b_ih1", "b_hh1"):
        assert np.abs(np.asarray(ii[bn])).max() == 0.0, f"nonzero bias {bn}"
    return d


def kernel(**inputs):
    t_steps = np.asarray(inputs["ref_tokens"]).shape[0]
    nc = build_program(t_steps)
    in_maps = [prep_core_inputs(inputs, c, t_steps) for c in range(NCORES)]
    res = run_bass_kernel_spmd(nc, in_maps, list(range(NCORES)))
    out = np.zeros((t_steps, B, V), np.float32)
    for c in range(NCORES):
        out[:, c * BL:(c + 1) * BL, :] = res.results[c]["y"]
    return out


if __name__ == "__main__":
    pass

